# revision 4
# baseline (speedup 1.0000x reference)
"""DADMM unrolled kernel for trn2 (8 NeuronCores, data-parallel over N).

Coefficient-space method: for each sample n, all iterates a_p[n,:], mu_p[n,:]
remain in span{X_q[n,:], a0_q[n,:] (, mu0_q[n,:])}.  The 15 ADMM iterations are
run on 8/12-dim coefficient vectors per sample; one Gram precompute and one
reconstruction touch the D=784 axis.
"""
import numpy as np

P = 4
D = 784
NCORES = 8
SLOT_OF_NODE = {0: 0, 2: 1, 1: 2, 3: 3}   # partition-block slot per node
NODE_OF_SLOT = [0, 2, 1, 3]
GROUPS = [(0, 2), (1, 3)]                 # color groups (update order)
PAIR = {0: (1, 3), 2: (1, 3), 1: (0, 2), 3: (0, 2)}  # neighbors per node

_CACHE = {}


def _rearr_scal(arr, Nloc):
    """[4, Nloc] -> scalar-layout [128, Nloc//32] (partition=(slot,c,n32), free=n8)."""
    nch = Nloc // 128
    r = arr.reshape(P, nch, 16, 8)
    out = np.stack([r[NODE_OF_SLOT[s]] for s in range(4)])  # [4,nch,16,8]
    return np.ascontiguousarray(out.reshape(4 * nch * 16, 8).astype(np.float32))


def _unrearr_scal(arr, Nloc):
    """inverse of _rearr_scal: [128, 8] -> [4, Nloc]"""
    nch = Nloc // 128
    r = np.asarray(arr).reshape(4, nch, 16, 8)
    out = np.empty((P, Nloc), dtype=arr.dtype)
    for s in range(4):
        out[NODE_OF_SLOT[s]] = r[s].reshape(Nloc)
    return out


def _build_wmats():
    """PE selector weights for cross-node neighbor sums (f32, 0/1)."""
    wg0 = np.zeros((128, 64), np.float32)   # out rows = G0 block; src nodes 1,3
    wg1 = np.zeros((128, 64), np.float32)   # out rows = G1 block; src nodes 0,2
    wall = np.zeros((128, 128), np.float32)
    for m in range(64):
        wg0[64 + (m % 32), m] = 1.0
        wg0[96 + (m % 32), m] = 1.0
        wg1[(m % 32), m] = 1.0
        wg1[32 + (m % 32), m] = 1.0
    for m in range(128):
        s = m // 32
        srcs = (2, 3) if s in (0, 1) else (0, 1)
        for src in srcs:
            wall[src * 32 + (m % 32), m] = 1.0
    return np.concatenate([wg0, wg1], axis=1), wall   # [128,128], [128,128]


def _build_hpat(habs, K):
    """per-partition hyp-constant patterns [128, K*9] in scalar layout."""
    NROLE = 9
    hp = np.zeros((128, K * NROLE), np.float32)
    for k in range(K):
        for s in range(4):
            p = NODE_OF_SLOT[s]
            h0, h1, h2, h3, h4 = habs[k, p]
            rows = slice(s * 32, (s + 1) * 32)
            c = k * NROLE
            hp[rows, c + 0] = h0 * h1          # c1: cS scale
            hp[rows, c + 1] = -2.0 * h1        # c2: cM scale
            hp[rows, c + 2] = 1.0 - 2.0 * h0 * h1  # c3: cA self scale
            hp[rows, c + 3] = -h1              # c4: resid one-hot
            hp[rows, c + 4] = 2.0 * h2         # c5: omega self
            hp[rows, c + 5] = -h2              # c6: nb-omega scale
            hp[rows, c + 6] = -h4              # c7: omega step
            hp[rows, c + 7] = h3               # c8: mu step
            hp[rows, c + 8] = 2.0 * h4         # c9: lambda(2x) step
    return hp


def _build_nc(NB, habs, K, Nloc):
    """Build the per-core Bass program.  NB = 8 (mu0==0) or 12."""
    from concourse import bacc, tile, mybir

    f32 = mybir.dt.float32
    AL = mybir.AluOpType
    AF = mybir.ActivationFunctionType
    nch = Nloc // 128            # chunks of 128 samples
    NW = 8 * NB                  # free width of coeff tiles (n8, j)
    NROLE = 9

    nc = bacc.Bacc("TRN2", target_bir_lowering=False, debug=False)
    xin = nc.dram_tensor("xin", [P, Nloc, D], f32, kind="ExternalInput").ap()
    a0in = nc.dram_tensor("a0in", [P, Nloc, D], f32, kind="ExternalInput").ap()
    if NB == 12:
        mu0in = nc.dram_tensor("mu0in", [P, Nloc, D], f32, kind="ExternalInput").ap()
    scal = nc.dram_tensor("scal", [128, 32], f32, kind="ExternalInput").ap()
    hpat_d = nc.dram_tensor("hpat", [128, K * NROLE], f32, kind="ExternalInput").ap()
    wg_d = nc.dram_tensor("wg", [128, 128], f32, kind="ExternalInput").ap()
    wall_d = nc.dram_tensor("wall", [128, 128], f32, kind="ExternalInput").ap()
    aout = nc.dram_tensor("aout", [P, Nloc, D], f32, kind="ExternalOutput").ap()
    omout = nc.dram_tensor("omout", [128, 8], f32, kind="ExternalOutput").ap()
    # internal DRAM bounce buffers for layout conversion
    gbounce = nc.dram_tensor("gbounce", [P, nch, 128, NB], f32)
    cabounce = nc.dram_tensor("cabounce", [128, NW], f32)

    with tile.TileContext(nc) as tc:
        with (
            tc.tile_pool(name="big", bufs=1) as big,
            tc.tile_pool(name="st", bufs=1) as st,
            tc.tile_pool(name="scr", bufs=3) as scr,
            tc.tile_pool(name="ps", bufs=2, space="PSUM") as ps,
        ):
            # ---------------- persistent tiles ----------------
            B = [[None] * nch for _ in range(NB)]     # basis [128, D] per (j, c)
            for q in range(P):
                for c in range(nch):
                    B[q][c] = big.tile([128, D], f32, tag=f"bx{q}{c}", name=f"bx{q}{c}")
                    nc.sync.dma_start(out=B[q][c][:], in_=xin[q, c * 128:(c + 1) * 128, :])
            for q in range(P):
                for c in range(nch):
                    B[4 + q][c] = big.tile([128, D], f32, tag=f"ba{q}{c}", name=f"ba{q}{c}")
                    nc.sync.dma_start(out=B[4 + q][c][:], in_=a0in[q, c * 128:(c + 1) * 128, :])
            if NB == 12:
                for q in range(P):
                    for c in range(nch):
                        B[8 + q][c] = big.tile([128, D], f32, tag=f"bm{q}{c}", name=f"bm{q}{c}")
                        nc.sync.dma_start(out=B[8 + q][c][:], in_=mu0in[q, c * 128:(c + 1) * 128, :])

            sc = st.tile([128, 32], f32)          # om~0, lt0, Ydual, y
            hp = st.tile([128, K * NROLE], f32)
            wg = st.tile([128, 128], f32)
            wall = st.tile([128, 128], f32)
            nc.sync.dma_start(out=sc[:], in_=scal)
            nc.sync.dma_start(out=hp[:], in_=hpat_d)
            nc.sync.dma_start(out=wg[:], in_=wg_d)
            nc.sync.dma_start(out=wall[:], in_=wall_d)

            S = st.tile([128, NW + 8], f32)       # cA (0:NW) | omega~ (NW:NW+8)
            cM = st.tile([128, NW], f32)
            Gs = st.tile([128, NW], f32)          # Gram, scalar layout
            Gc = [[st.tile([128, NB], f32, tag=f"gc{q}{c}", name=f"gc{q}{c}") for c in range(nch)]
                  for q in range(P)]              # Gram, coeff layout
            xta = st.tile([128, 8], f32)
            resid = st.tile([128, 8], f32)

            # ---------------- Gram (coeff layout) ----------------
            # G[q][c][:, j] = rowsum(X_q[c] * B_j[c]); split across two lanes.
            lane = 0
            for c in range(nch):
                for q in range(P):
                    for j in range(NB):
                        if j < 4 and j < q:
                            # symmetric X-X entry, copy from Gc[j][c][:, q]
                            nc.scalar.copy(Gc[q][c][:, j:j + 1], Gc[j][c][:, q:q + 1])
                            continue
                        if lane == 0:
                            g_scr = scr.tile([128, D], f32, tag="gscr_v")
                            nc.vector.scalar_tensor_tensor(
                                out=g_scr[:], in0=B[q][c][:], scalar=1.0,
                                in1=B[j][c][:], op0=AL.mult, op1=AL.mult,
                                accum_out=Gc[q][c][:, j:j + 1])
                        else:
                            g_scr = scr.tile([128, D], f32, tag="gscr_g")
                            g_scr2 = scr.tile([128, D], f32, tag="gscr_g2")
                            nc.gpsimd.tensor_tensor(
                                out=g_scr[:], in0=B[q][c][:], in1=B[j][c][:],
                                op=AL.mult)
                            nc.scalar.activation(
                                out=g_scr2[:], in_=g_scr[:], func=AF.Copy,
                                accum_out=Gc[q][c][:, j:j + 1])
                        lane ^= 1

            # bounce Gram to scalar layout via DRAM
            for q in range(P):
                for c in range(nch):
                    s = SLOT_OF_NODE[q]
                    nc.sync.dma_start(out=gbounce.ap()[s, c], in_=Gc[q][c][:])
            gview = gbounce.ap().rearrange(
                "s c (n32 n8) j -> (s c n32) (n8 j)", n8=8)
            nc.sync.dma_start(out=Gs[:], in_=gview)

            # ---------------- state init ----------------
            nc.vector.memset(S[:, 0:NW], 0.0)
            nc.gpsimd.tensor_copy(S[:, NW:NW + 8], sc[:, 0:8])      # omega~0
            lt = st.tile([128, 8], f32)
            nc.gpsimd.tensor_copy(lt[:], sc[:, 8:16])               # 2*lambda0
            nc.vector.memset(cM[:], 0.0)
            caview = S[:, 0:NW].rearrange("p (a b) -> p a b", b=NB)
            cmview = cM.rearrange("p (a b) -> p a b", b=NB)
            for s in range(4):
                p = NODE_OF_SLOT[s]
                rows = slice(s * 32, (s + 1) * 32)
                nc.vector.memset(caview[rows, :, 4 + p], 1.0)
                if NB == 12:
                    nc.vector.memset(cmview[rows, :, 8 + p], 1.0)

            # ---------------- iterations ----------------
            gsv = Gs.rearrange("p (a b) -> p a b", b=NB)
            for k in range(K):
                def hc(r, rws=slice(0, 128)):
                    return hp[rws, k * NROLE + r: k * NROLE + r + 1]
                for gi, grp in enumerate(GROUPS):
                    lo, hi = gi * 64, gi * 64 + 64
                    rows = slice(lo, hi)
                    # neighbor sums on PE -> PSUM [64, NW+8]
                    nb = ps.tile([64, NW + 8], f32, tag="nbg")
                    nc.tensor.matmul(nb[:], wg[:, gi * 64:(gi + 1) * 64], S[:, :])
                    # xta = rowblock-sum_j(cA * G)
                    prod = scr.tile([128, NW], f32, tag="prod")
                    nc.gpsimd.tensor_tensor(out=prod[rows, :], in0=S[rows, 0:NW],
                                            in1=Gs[rows, :], op=AL.mult)
                    pv = prod.rearrange("p (a b) -> p a b", b=NB)
                    nc.vector.tensor_reduce(out=xta[rows, :], in_=pv[rows],
                                            axis=mybir.AxisListType.X, op=AL.add)
                    # resid = xta + omega~
                    nc.vector.scalar_tensor_tensor(
                        out=resid[rows, :], in0=xta[rows, :], scalar=1.0,
                        in1=S[rows, NW:NW + 8], op0=AL.mult, op1=AL.add)
                    # cA update: u = c2*cM ; u += c1*cS ; cA = c3*cA + u
                    u = scr.tile([128, NW], f32, tag="ua")
                    nc.scalar.activation(out=u[rows, :], in_=cM[rows, :],
                                         func=AF.Copy, scale=hc(1, rows))
                    nc.vector.scalar_tensor_tensor(
                        out=u[rows, :], in0=nb[:, 0:NW], scalar=hc(0, rows),
                        in1=u[rows, :], op0=AL.mult, op1=AL.add)
                    nc.vector.scalar_tensor_tensor(
                        out=S[rows, 0:NW], in0=S[rows, 0:NW], scalar=hc(2, rows),
                        in1=u[rows, :], op0=AL.mult, op1=AL.add)
                    # one-hot resid injection (col j=p of each node block)
                    for p in grp:
                        s = SLOT_OF_NODE[p]
                        rr = slice(s * 32, (s + 1) * 32)
                        nc.vector.scalar_tensor_tensor(
                            out=caview[rr, :, p], in0=resid[rr, :],
                            scalar=hc(3, rr), in1=caview[rr, :, p],
                            op0=AL.mult, op1=AL.add)
                    # omega update
                    u1 = scr.tile([128, 8], f32, tag="u1")
                    u2 = scr.tile([128, 8], f32, tag="u2")
                    nc.vector.scalar_tensor_tensor(
                        out=u1[rows, :], in0=S[rows, NW:NW + 8], scalar=hc(4, rows),
                        in1=resid[rows, :], op0=AL.mult, op1=AL.add)
                    # v = nb_omega - Ydual ; u2 = c6*v + lt
                    nc.vector.scalar_tensor_tensor(
                        out=u2[rows, :], in0=nb[:, NW:NW + 8], scalar=1.0,
                        in1=sc[rows, 16:24], op0=AL.mult, op1=AL.subtract)
                    nc.vector.scalar_tensor_tensor(
                        out=u2[rows, :], in0=u2[rows, :], scalar=hc(5, rows),
                        in1=lt[rows, :], op0=AL.mult, op1=AL.add)
                    u3 = scr.tile([128, 8], f32, tag="u3")
                    nc.gpsimd.tensor_tensor(out=u3[rows, :], in0=u1[rows, :],
                                            in1=u2[rows, :], op=AL.add)
                    nc.vector.scalar_tensor_tensor(
                        out=S[rows, NW:NW + 8], in0=u3[rows, :], scalar=hc(6, rows),
                        in1=S[rows, NW:NW + 8], op0=AL.mult, op1=AL.add)
                # dual updates
                nball = ps.tile([128, NW + 8], f32, tag="nball")
                nc.tensor.matmul(nball[:], wall[:], S[:, :])
                v = scr.tile([128, NW], f32, tag="vd")
                nc.vector.scalar_tensor_tensor(
                    out=v[:], in0=S[:, 0:NW], scalar=2.0, in1=nball[:, 0:NW],
                    op0=AL.mult, op1=AL.subtract)
                nc.vector.scalar_tensor_tensor(
                    out=cM[:], in0=v[:], scalar=hc(7), in1=cM[:],
                    op0=AL.mult, op1=AL.add)
                w = scr.tile([128, 8], f32, tag="wd")
                nc.vector.scalar_tensor_tensor(
                    out=w[:], in0=S[:, NW:NW + 8], scalar=2.0,
                    in1=nball[:, NW:NW + 8], op0=AL.mult, op1=AL.subtract)
                w2 = scr.tile([128, 8], f32, tag="wd2")
                nc.gpsimd.tensor_tensor(out=w2[:], in0=w[:], in1=sc[:, 16:24],
                                        op=AL.add)
                nc.vector.scalar_tensor_tensor(
                    out=lt[:], in0=w2[:], scalar=hc(8), in1=lt[:],
                    op0=AL.mult, op1=AL.add)

            # ---------------- outputs ----------------
            # omega = omega~ + y
            omt = st.tile([128, 8], f32)
            nc.gpsimd.tensor_tensor(out=omt[:], in0=S[:, NW:NW + 8],
                                    in1=sc[:, 24:32], op=AL.add)
            nc.sync.dma_start(out=omout, in_=omt[:])

            # bounce cA to coeff layout, then reconstruct a = sum_j cA_j * B_j
            nc.sync.dma_start(out=cabounce.ap(), in_=S[:, 0:NW])
            cav = cabounce.ap().rearrange(
                "(s c n32) (n8 j) -> (s c) (n32 n8) j", c=nch, n32=16, j=NB)
            for p in range(P):
                sl = SLOT_OF_NODE[p]
                for c in range(nch):
                    capc = st.tile([128, NB], f32, tag=f"ca{p}{c}")
                    nc.sync.dma_start(out=capc[:], in_=cav[sl * nch + c])
                    accv = scr.tile([128, D], f32, tag="accv")
                    accg = scr.tile([128, D], f32, tag="accg")
                    # DVE lane: terms 0..NBV-1 ; gpsimd lane: rest
                    NBV = NB - 3
                    nc.vector.tensor_scalar_mul(accv[:], B[0][c][:], capc[:, 0:1])
                    for j in range(1, NBV):
                        nc.vector.scalar_tensor_tensor(
                            out=accv[:], in0=B[j][c][:], scalar=capc[:, j:j + 1],
                            in1=accv[:], op0=AL.mult, op1=AL.add)
                    nc.gpsimd.tensor_scalar_mul(accg[:], B[NBV][c][:],
                                                capc[:, NBV:NBV + 1])
                    for j in range(NBV + 1, NB):
                        gt = scr.tile([128, D], f32, tag="gt")
                        nc.gpsimd.tensor_scalar_mul(gt[:], B[j][c][:],
                                                    capc[:, j:j + 1])
                        nc.gpsimd.tensor_tensor(out=accg[:], in0=accg[:],
                                                in1=gt[:], op=AL.add)
                    nc.vector.tensor_tensor(out=accv[:], in0=accv[:],
                                            in1=accg[:], op=AL.add)
                    nc.sync.dma_start(out=aout[p, c * 128:(c + 1) * 128, :],
                                      in_=accv[:])
    nc.compile()
    return nc


def kernel(inputs, labels, a0, omega0, mu0, lamda0, hyp, MAX_ITER):
    inputs = np.ascontiguousarray(np.asarray(inputs, dtype=np.float32))
    labels = np.asarray(labels, dtype=np.float32)
    a0 = np.ascontiguousarray(np.asarray(a0, dtype=np.float32))
    omega0 = np.asarray(omega0, dtype=np.float32)
    mu0 = np.ascontiguousarray(np.asarray(mu0, dtype=np.float32))
    lamda0 = np.asarray(lamda0, dtype=np.float32)
    K = int(MAX_ITER)
    habs = np.abs(np.asarray(hyp, dtype=np.float32))[:K]

    N = inputs.shape[1]
    Nloc = N // NCORES
    NB = 8 if not np.any(mu0) else 12

    key = (NB, K, Nloc, habs.tobytes())
    if key not in _CACHE:
        _CACHE.clear()
        _CACHE[key] = _build_nc(NB, habs, K, Nloc)
    nc = _CACHE[key]

    wg, wall = _build_wmats()
    in_maps = []
    for i in range(NCORES):
        sl = slice(i * Nloc, (i + 1) * Nloc)
        y = labels[:, sl]
        om = omega0[:, sl]
        lam = lamda0[:, sl]
        ydual = np.empty_like(y)
        for p in range(P):
            q1, q2 = PAIR[p]
            ydual[p] = 2.0 * y[p] - y[q1] - y[q2]
        scal = np.concatenate([
            _rearr_scal(om - y, Nloc), _rearr_scal(2.0 * lam, Nloc),
            _rearr_scal(ydual, Nloc), _rearr_scal(y, Nloc)], axis=1)
        m = {
            "xin": inputs[:, sl, :], "a0in": a0[:, sl, :],
            "scal": scal, "hpat": _build_hpat(habs, K),
            "wg": wg, "wall": wall,
        }
        if NB == 12:
            m["mu0in"] = mu0[:, sl, :]
        in_maps.append(m)

    from concourse.bass_utils import run_bass_kernel_spmd
    res = run_bass_kernel_spmd(nc, in_maps, list(range(NCORES)))

    a_full = np.empty((P, N, D), dtype=np.float32)
    om_full = np.empty((P, N), dtype=np.float32)
    for i in range(NCORES):
        sl = slice(i * Nloc, (i + 1) * Nloc)
        a_full[:, sl, :] = res.results[i]["aout"]
        om_full[:, sl] = _unrearr_scal(res.results[i]["omout"], Nloc)
    return a_full, om_full
